# revision 16
# baseline (speedup 1.0000x reference)
"""Trainium2 Bass kernel for ErnieImageAttention (non-causal MHA with per-head
RMSNorm on q/k + rotary embedding), tensor-parallel over heads on 8 NeuronCores.

Sharding: 16 heads / 8 cores = 2 heads per core. Each core computes its heads'
q/k/v projections, attention, and a partial output projection (row-parallel
Wo); the host sums the 8 partials and adds the bias.

Per-core dataflow (S=4096, D=2048, Hd=128, 2 local heads):
  phase 1 (per 128-row s-tile):
    qkv = hiddenT-chunk matmuls (bf16, fused moving operand N=768 = q|k|v for
    both heads -> 16 matmuls + 16 weight loads per tile) accumulated in PSUM;
    RMSNorm stats via Square-with-accum on ACT; RoPE via host-precomputed
    coefficient tables (g gains and the 1/sqrt(Hd) logit scale folded in);
    PE-transpose q/k 128x128 bf16 tiles into [d, s] layout; v kept [s, d].
  phase 2 (per 512-col q-block, per head, k in groups of 3(A)/2(B) tiles):
    scoresT[k,q] = kT.T @ qT (bf16) into a 3-bank A / 2-bank B PSUM ping-pong
    one wide Exp per group on ACT -- ACT does ONLY exps in phase 2
    denominator: folded into accA lanes by DVE bf16 adds; the deferred tail
    does 3 ones-matmuls + reciprocal + rank-1 broadcast into a DEDICATED
    1-bank PSUM slot (no WAR against the score regions)
    attn_T[d,q] = sum_k V[k,d]^T expT[k,q] accumulated in PSUM (bf16);
    po is evicted to SBUF by DVE (araw) right after the k-loop so the po/pf
    bank rotation never blocks; normalize in the deferred tail (DVE)
  phase 3 (inline per q-block): fin[s, :2048] = sum_h attnT_h.T @ WoT_h (f32r)

Softmax is max-subtraction-free: logits are ~N(0,1) by construction
(RMSNorm'd q/k, 1/sqrt(Hd) folded into q's rope tables).
"""

import numpy as np
import ml_dtypes

import concourse.bass as bass
import concourse.tile as tile
from concourse import bacc, mybir
from concourse import bass_utils
from concourse.masks import make_identity

F32 = mybir.dt.float32
F32R = mybir.dt.float32r
BF16 = mybir.dt.bfloat16
AX = mybir.AxisListType
AF = mybir.ActivationFunctionType

S = 4096
D = 2048
HD = 128
HEADS = 16
NCORES = 8
HLOC = HEADS // NCORES  # 2 heads per core
DLOC = HLOC * HD  # 256 local head dims
NQKV = 3 * DLOC  # fused q|k|v projection width
CH = D // 128  # 16 contraction chunks for projections
EPS = 1e-5
SCL = 1.0 / np.sqrt(HD)

QCOLS = 512  # q columns per attention block
KGA = 3  # k tiles per A (3-bank) score group
KGB = 2  # k tiles per B (2-bank) score group


def build(nc, tc, io, s_len):
    st_n = s_len // 128  # s tiles
    qb_n = s_len // QCOLS  # q blocks
    qb_st = QCOLS // 128  # s tiles per q block
    kt_n = st_n  # k tiles

    ht, wqkv, wo, cgq, sgq, cgk, sgk, out = (
        io["ht"], io["wqkv"], io["wo"],
        io["cgq"], io["sgq"], io["cgk"], io["sgk"], io["out"],
    )

    import contextlib

    with contextlib.ExitStack() as ctx:
        ctx.enter_context(nc.allow_low_precision(
            reason="bf16/f32r compute; values are O(1) and the rel-err "
                   "budget is 2e-2"))
        consts = ctx.enter_context(tc.tile_pool(name="consts", bufs=1))
        persist = ctx.enter_context(tc.tile_pool(name="persist", bufs=1))
        ht_pool = ctx.enter_context(tc.tile_pool(name="ht", bufs=3))
        cs_pool = ctx.enter_context(tc.tile_pool(name="cs", bufs=3))
        work = ctx.enter_context(tc.tile_pool(name="work", bufs=2))
        et_pool = ctx.enter_context(tc.tile_pool(name="et", bufs=4))
        at_pool = ctx.enter_context(tc.tile_pool(name="at", bufs=6))
        araw_pool = ctx.enter_context(tc.tile_pool(name="araw", bufs=3))
        acc_pool = ctx.enter_context(tc.tile_pool(name="acc", bufs=2))
        rc_pool = ctx.enter_context(tc.tile_pool(name="rc", bufs=2))
        fin_pool = ctx.enter_context(tc.tile_pool(name="fin", bufs=6))

        # constants
        ident = consts.tile([128, 128], BF16)
        make_identity(nc, ident[:])
        ones_f32 = consts.tile([128, 1], F32)
        nc.vector.memset(ones_f32[:], 1.0)
        ones_col_bf = consts.tile([128, 1], BF16)
        nc.vector.tensor_copy(ones_col_bf[:], ones_f32[:])
        ones_row_f32 = consts.tile([1, 128], F32)
        nc.vector.memset(ones_row_f32[:], 1.0)
        ones_row = consts.tile([1, 128], F32R)
        nc.vector.tensor_copy(ones_row[:], ones_row_f32[:])
        eps_t = consts.tile([128, 1], F32)
        nc.vector.memset(eps_t[:], EPS)

        # startup: first ht tile ahead of the weight burst, two more right
        # behind it, so the projection pipeline never waits on the SP ring
        # while the coef stream catches up
        ht_first = {}
        for st0 in range(1):
            for tag, cs0 in (("hta", 0), ("htb", CH // 2)):
                t = ht_pool.tile([128, CH // 2, 128], BF16, tag=tag,
                                 name=f"ht{st0}{tag}")
                nc.sync.dma_start(out=t[:],
                                  in_=ht[st0][:, cs0:cs0 + CH // 2, :])
                ht_first[(st0, tag)] = t

        # weights right behind the first ht tile, split fine so the first
        # projection chunks unblock as early as possible; wo on the idle ACT
        # ring (needed only by the out-projection, ~qb1)
        w_sb = []
        for part in range(8):
            t = consts.tile([128, 2, NQKV], BF16, name=f"wqkv{part}")
            nc.sync.dma_start(
                out=t[:], in_=wqkv[:, part * 2:(part + 1) * 2, :])
            w_sb.append(t)
        wo_sb = consts.tile([128, HLOC, D], F32R)
        nc.scalar.dma_start(out=wo_sb[:], in_=wo)

        for st0 in range(1, 3):
            for tag, cs0 in (("hta", 0), ("htb", CH // 2)):
                t = ht_pool.tile([128, CH // 2, 128], BF16, tag=tag,
                                 name=f"ht{st0}{tag}")
                nc.sync.dma_start(out=t[:],
                                  in_=ht[st0][:, cs0:cs0 + CH // 2, :])
                ht_first[(st0, tag)] = t

        # persistent per-head transposed q/k and v
        qT_sb = persist.tile([128, HLOC, st_n, 128], BF16)
        kT_sb = persist.tile([128, HLOC, st_n, 128], BF16)
        v_sb = persist.tile([128, st_n, DLOC], BF16)

        # ---------------- phase 1: projections + norm + rope + transpose ----
        with tc.tile_pool(name="ps1", bufs=2, space="PSUM") as ps1:
            for st in range(st_n):
                ss = slice(st * 128, (st + 1) * 128)
                if st < 3:
                    ht_a = ht_first[(st, "hta")]
                    ht_b = ht_first[(st, "htb")]
                else:
                    ht_a = ht_pool.tile([128, CH // 2, 128], BF16, tag="hta")
                    nc.sync.dma_start(out=ht_a[:], in_=ht[st][:, 0:CH // 2, :])
                    ht_b = ht_pool.tile([128, CH // 2, 128], BF16, tag="htb")
                    nc.sync.dma_start(out=ht_b[:], in_=ht[st][:, CH // 2:, :])
                cgq_t = cs_pool.tile([128, DLOC], BF16, tag="cgq")
                nc.sync.dma_start(out=cgq_t[:], in_=cgq[ss, :])
                sgq_t = cs_pool.tile([128, DLOC], BF16, tag="sgq")
                nc.sync.dma_start(out=sgq_t[:], in_=sgq[ss, :])
                cgk_t = cs_pool.tile([128, DLOC], BF16, tag="cgk")
                nc.sync.dma_start(out=cgk_t[:], in_=cgk[ss, :])
                sgk_t = cs_pool.tile([128, DLOC], BF16, tag="sgk")
                nc.sync.dma_start(out=sgk_t[:], in_=sgk[ss, :])

                pp = ps1.tile([128, NQKV], F32, tag="pp")
                for c in range(CH):
                    lhs = (ht_a if c < CH // 2 else ht_b)[:, c % (CH // 2), :]
                    # moving operand ISA limit is 512 elements: q|k then v
                    nc.tensor.matmul(pp[:, 0:512], lhs,
                                     w_sb[c // 2][:, c % 2, 0:512],
                                     start=(c == 0), stop=(c == CH - 1))
                    nc.tensor.matmul(pp[:, 512:NQKV], lhs,
                                     w_sb[c // 2][:, c % 2, 512:NQKV],
                                     start=(c == 0), stop=(c == CH - 1))
                pq = pp[:, 0:DLOC]
                pk = pp[:, DLOC:2 * DLOC]
                pv = pp[:, 2 * DLOC:3 * DLOC]

                # v: PSUM -> SBUF bf16
                nc.scalar.copy(v_sb[:, st, :], pv)

                # rms stats: Square with free-dim accumulation -> sum(q^2)
                varq = work.tile([128, HLOC], F32, tag="varq")
                vark = work.tile([128, HLOC], F32, tag="vark")
                sqd = work.tile([128, HD], F32, tag="sqd")  # dump
                for h in range(HLOC):
                    hs = slice(h * HD, (h + 1) * HD)
                    nc.scalar.activation(sqd[:], pq[:, hs], AF.Square,
                                         accum_out=varq[:, h:h + 1])
                    nc.scalar.activation(sqd[:], pk[:, hs], AF.Square,
                                         accum_out=vark[:, h:h + 1])
                sigq = work.tile([128, HLOC], F32, tag="sigq")
                nc.scalar.activation(sigq[:], varq[:], AF.Sqrt,
                                     bias=eps_t[:], scale=1.0 / HD)
                rq = work.tile([128, HLOC], F32, tag="rq")
                nc.vector.reciprocal_approx_fast(rq[:], sigq[:])
                sigk = work.tile([128, HLOC], F32, tag="sigk")
                nc.scalar.activation(sigk[:], vark[:], AF.Sqrt,
                                     bias=eps_t[:], scale=1.0 / HD)
                rk = work.tile([128, HLOC], F32, tag="rk")
                nc.vector.reciprocal_approx_fast(rk[:], sigk[:])

                # rope: out = (r*x) . CG + shift64(r*x) . SG   (per tensor)
                for name, psrc, r, cg, sg, dstT in (
                    ("q", pq, rq, cgq_t, sgq_t, qT_sb),
                    ("k", pk, rk, cgk_t, sgk_t, kT_sb),
                ):
                    xs = work.tile([128, DLOC], F32, tag=f"xs{name}")
                    for h in range(HLOC):
                        hs = slice(h * HD, (h + 1) * HD)
                        nc.vector.tensor_scalar_mul(xs[:, hs], psrc[:, hs],
                                                    r[:, h:h + 1])
                    m1 = work.tile([128, DLOC], F32, tag=f"m1{name}")
                    nc.vector.tensor_mul(m1[:], xs[:], cg[:])
                    m2 = work.tile([128, DLOC], F32, tag=f"m2{name}")
                    x4 = xs[:].rearrange("p (h t u) -> p h t u", h=HLOC, t=2)
                    m4 = m2[:].rearrange("p (h t u) -> p h t u", h=HLOC, t=2)
                    g4 = sg[:].rearrange("p (h t u) -> p h t u", h=HLOC, t=2)
                    nc.vector.tensor_mul(m4[:, :, 0, :], x4[:, :, 1, :],
                                         g4[:, :, 0, :])
                    nc.vector.tensor_mul(m4[:, :, 1, :], x4[:, :, 0, :],
                                         g4[:, :, 1, :])
                    xa = work.tile([128, DLOC], BF16, tag=f"xa{name}")
                    nc.vector.tensor_add(xa[:], m1[:], m2[:])
                    for h in range(HLOC):
                        hs = slice(h * HD, (h + 1) * HD)
                        ptp = ps1.tile([128, 128], BF16, tag="ptp")
                        nc.tensor.transpose(ptp[:], xa[:, hs], ident[:])
                        nc.scalar.copy(dstT[:, h, st, :], ptp[:])

        # ---------------- phase 2+3: attention + output projection ----------
        # The two heads' score->exp->PV chains are interleaved step by step:
        # while ACT runs one head's exp, PE runs the other head's matmuls, so
        # every cross-engine dependency has a full step (~2.3us) of slack and
        # semaphore latency never stalls the in-order PE queue.
        # PSUM: sc_h0(2) + sc_h1(2) + po_h0(1) + po_h1(1) + pf(1) + pd(1) = 8.
        KG = 2  # k tiles per score group
        kg_n = kt_n // KG
        with (
            tc.tile_pool(name="psS", bufs=1, space="PSUM") as psS,
            tc.tile_pool(name="psP", bufs=1, space="PSUM") as psP,
            tc.tile_pool(name="psF", bufs=1, space="PSUM") as psF,
            tc.tile_pool(name="psD", bufs=1, space="PSUM") as psD,
        ):
            pending = []  # deferred tails + out-proj chunks

            def outproj_chunks(qb, ats):
                chunks = []
                for sti in range(qb_st):
                    st = qb * qb_st + sti
                    sl = slice(sti * 128, (sti + 1) * 128)
                    for nchunk in range(D // 512):
                        ns = slice(nchunk * 512, (nchunk + 1) * 512)

                        def emit(st=st, sl=sl, ns=ns, ats=ats):
                            pf = psF.tile([128, QCOLS], F32, tag="pf",
                                          name=f"pf_{st}_{ns.start}")
                            for h in range(HLOC):
                                nc.tensor.matmul(pf[:], ats[h][:, sl],
                                                 wo_sb[:, h, ns],
                                                 start=(h == 0),
                                                 stop=(h == HLOC - 1))
                            fin = fin_pool.tile([128, 512], F32R, tag="fin")
                            nc.vector.tensor_copy(fin[:], pf[:])
                            nc.sync.dma_start(
                                out=out[st * 128:(st + 1) * 128, ns],
                                in_=fin[:])
                        chunks.append(emit)
                return chunks

            for qb in range(qb_n):
                q_rhs = [qT_sb[:, h, qb * qb_st:(qb + 1) * qb_st, :]
                         for h in range(HLOC)]
                accA = [acc_pool.tile([128, KG, QCOLS], BF16,
                                      tag=f"accA{h}", name=f"accA{h}")
                        for h in range(HLOC)]
                po = [psP.tile([128, QCOLS], F32, tag=f"po{h}",
                               name=f"po{h}") for h in range(HLOC)]
                ets = {}

                def flush(h, g):
                    et = ets.pop((h, g))
                    acc = accA[h][:, :, :]
                    if g == 0:
                        nc.vector.tensor_copy(acc, et[:])
                    else:
                        nc.vector.tensor_add(acc, acc, et[:])
                    for j in range(KG):
                        kt = g * KG + j
                        nc.tensor.matmul(po[h][:],
                                         v_sb[:, kt, h * HD:(h + 1) * HD],
                                         et[:, j, :], start=(kt == 0),
                                         stop=(kt == kt_n - 1))

                for g in range(kg_n):
                    for h in range(HLOC):
                        sc = psS.tile([128, KG, QCOLS], F32, tag=f"sc{h}",
                                      name=f"sc{h}")
                        for j in range(KG):
                            nc.tensor.matmul(sc[:, j, :],
                                             kT_sb[:, h, g * KG + j, :],
                                             q_rhs[h], start=True, stop=True)
                        et = et_pool.tile([128, KG, QCOLS], BF16, tag="et")
                        nc.scalar.activation(et[:], sc[:], AF.Exp)
                        ets[(h, g)] = et
                    if g >= 1:
                        for h in range(HLOC):
                            flush(h, g - 1)
                    if pending:
                        pending.pop(0)()
                    if g % 8 == 7 and pending:
                        pending.pop(0)()
                for h in range(HLOC):
                    flush(h, kg_n - 1)

                # evict po now (DVE) so next qb's PV chains don't wait on the
                # deferred tails; ACT's FIFO stays exps-only
                ats = []
                for h in range(HLOC):
                    araw = araw_pool.tile([128, QCOLS], F32R, tag="araw")
                    nc.vector.tensor_copy(araw[:], po[h][:])
                    at = at_pool.tile([128, QCOLS], F32R, tag="at")
                    ats.append(at)

                    def tail(accA=accA[h], araw=araw, at=at):
                        pdb = psD.tile([128, QCOLS], F32, tag="pd",
                                       name="pdb")
                        pd = pdb[0:1, :]
                        for i in range(KG):
                            nc.tensor.matmul(pd, ones_col_bf[:],
                                             accA[:, i, :],
                                             start=(i == 0),
                                             stop=(i == KG - 1))
                        rsb = rc_pool.tile([1, QCOLS], F32, tag="rsb")
                        nc.vector.reciprocal_approx_fast(rsb[:], pd)
                        rsr = rc_pool.tile([1, QCOLS], F32R, tag="rsr")
                        nc.vector.tensor_copy(rsr[:], rsb[:])
                        nc.tensor.matmul(pdb[:], ones_row[:], rsr[:],
                                         start=True, stop=True)
                        nc.vector.tensor_mul(at[:], araw[:], pdb[:])
                    pending.insert(h, tail)

                pending += outproj_chunks(qb, ats)
            while pending:
                pending.pop(0)()


def build_program(s_len=S):
    nc = bacc.Bacc("TRN2", target_bir_lowering=False, debug=False,
                   enable_asserts=False)
    st_n = s_len // 128
    io = {
        "ht": nc.dram_tensor("ht", [st_n, 128, CH, 128], BF16,
                             kind="ExternalInput").ap(),
        "wqkv": nc.dram_tensor("wqkv", [128, CH, NQKV], BF16,
                               kind="ExternalInput").ap(),
        "wo": nc.dram_tensor("wo", [128, HLOC, D], F32R,
                             kind="ExternalInput").ap(),
        "cgq": nc.dram_tensor("cgq", [s_len, DLOC], BF16,
                              kind="ExternalInput").ap(),
        "sgq": nc.dram_tensor("sgq", [s_len, DLOC], BF16,
                              kind="ExternalInput").ap(),
        "cgk": nc.dram_tensor("cgk", [s_len, DLOC], BF16,
                              kind="ExternalInput").ap(),
        "sgk": nc.dram_tensor("sgk", [s_len, DLOC], BF16,
                              kind="ExternalInput").ap(),
        "out": nc.dram_tensor("out", [s_len, D], F32R,
                              kind="ExternalOutput").ap(),
    }
    with tile.TileContext(nc) as tc:
        build(nc, tc, io, s_len)
    nc.compile()
    return nc


def prep_inputs(inputs, s_len=S):
    """Host-side preprocessing: transposed/tiled bf16 layouts + rope
    coefficient tables (g gains and the 1/sqrt(Hd) scale folded in,
    duplicated per local head for full-width elementwise ops)."""
    bf16 = ml_dtypes.bfloat16
    hs = np.asarray(inputs["hidden_states"], np.float32).reshape(s_len, D)
    st_n = s_len // 128
    ht = np.ascontiguousarray(
        hs.reshape(st_n, 128, CH, 128).transpose(0, 3, 2, 1)).astype(bf16)

    fc = np.asarray(inputs["freqs_cis"], np.float32).reshape(s_len, HD)
    cos = np.cos(fc)
    sin = np.sin(fc)
    gq = np.asarray(inputs["gq"], np.float32)
    gk = np.asarray(inputs["gk"], np.float32)

    def coef(g, scale):
        cg = cos * g[None, :] * scale
        sg = np.empty_like(sin)
        sg[:, :64] = -sin[:, :64] * g[None, 64:] * scale
        sg[:, 64:] = sin[:, 64:] * g[None, :64] * scale
        cg2 = np.ascontiguousarray(np.tile(cg, (1, HLOC))).astype(bf16)
        sg2 = np.ascontiguousarray(np.tile(sg, (1, HLOC))).astype(bf16)
        return cg2, sg2

    cgq, sgq = coef(gq, SCL)
    cgk, sgk = coef(gk, 1.0)

    Wq = np.asarray(inputs["Wq"], np.float32)
    Wk = np.asarray(inputs["Wk"], np.float32)
    Wv = np.asarray(inputs["Wv"], np.float32)
    Wo = np.asarray(inputs["Wo"], np.float32)

    in_maps = []
    for c in range(NCORES):
        cols = slice(DLOC * c, DLOC * (c + 1))

        def wtile(W):
            # [D, DLOC] -> [128(part), CH, DLOC]
            return W[cols, :].T.reshape(CH, 128, DLOC).transpose(1, 0, 2)

        wqkv_c = np.ascontiguousarray(
            np.concatenate([wtile(Wq), wtile(Wk), wtile(Wv)],
                           axis=2)).astype(bf16)
        wo_c = np.ascontiguousarray(
            Wo[:, cols].T.reshape(HLOC, 128, D).transpose(1, 0, 2))
        in_maps.append({
            "ht": ht, "wqkv": wqkv_c, "wo": wo_c,
            "cgq": cgq, "sgq": sgq, "cgk": cgk, "sgk": sgk,
        })
    return in_maps


_CACHE = {}


def run_full(inputs, trace=False, **kw):
    if "nc" not in _CACHE:
        _CACHE["nc"] = build_program(S)
    nc = _CACHE["nc"]
    in_maps = prep_inputs(inputs, S)
    res = bass_utils.run_bass_kernel_spmd(
        nc, in_maps, core_ids=list(range(NCORES)), trace=trace, **kw)
    total = res.results[0]["out"].astype(np.float64)
    for c in range(1, NCORES):
        total += res.results[c]["out"]
    total += np.asarray(inputs["bo"], np.float64)[None, :]
    out = total.astype(np.float32).reshape(1, S, D)
    return out, res


def kernel(**inputs):
    out, _ = run_full(inputs, trace=False)
    return out
